# revision 62
# baseline (speedup 1.0000x reference)
"""Bezier-to-image Gaussian splat kernel for Trainium2 (8 NeuronCores).

Reference computation (per sample b of 256):
    T = warped cubic Bernstein basis (30, 4)
    points = einsum('nk,blkc->blnc', T, x.reshape(B,160,4,2))   # (B,160,30,2)
    gx[b,l,i,n] = exp(-(i/60 - X[b,l,n])^2 / 2e-4)
    out[b,i,j]  = min(sum_{l,n} gx[b,l,i,n]*gy[b,l,j,n], 1)     # (B,60,60)

Strategy: pure data parallel, 32 samples per core.  Host pre-transposes
the control points to k-major layout so the input lands in one contiguous
DMA (the old transposing DMA cost ~21k 8-byte descriptors ~ 200us), and
the output DRAM tensor is [W, BC, W] so the store is contiguous too (host
un-permutes).

Per-core pipeline: r = 60*X via tiny fp32 PE matmuls (4-sample units,
scheduled early so the in-order PE queue never blocks them behind image
matmuls); r converts to int16 fixed point (256*r) on DVE; the band
d[p, 120*cs+2*i+co] = 256*(iota - r) is one DVE tensor_tensor subtract
per sample whose operands are all 2-byte with step-1 innermost dims ->
DVE 2x_1P mode (2.6us/sample instead of 5.1); ScalarE (the bottleneck at
1 elem/cycle/lane, ~4.15us/sample) evaluates the Gaussian via
Derivative_Erf(SDERF/256 * d) IN PLACE (f16 over the i16 input via
bitcast - the ACT write trails its read, halving band SBUF) in one
instruction per sample group; the 60x60 image accumulates on PE over 40
interleaved-stride chunk matmuls in one PSUM bank; min(s*img,1) runs on
DVE (tensor_scalar mult+min) to keep ScalarE saturated.

Scheduling: ScalarE group sizes ramp 2x-half, 3x-single, 5x-pair, then
quads (the DVE needs 2.7us of subtract per sample vs ScalarE's 4.15us,
so big ACT groups are only safe once the DVE has built up surplus); each
group's subtracts are emitted one full iteration AHEAD of its ACT so the
ACT dependency is satisfied a window early and the DVE queue's mins and
rcopies can't head-of-line block a critical subtract; the last sample
drains in two half-band ACTs so its first image half overlaps its second
ACT.  Steady state is ~97% ScalarE-occupied; ~154us/kernel = ~13us fill
+ 133us ACT + ~2us gaps + ~6us drain.
"""

import math

import numpy as np
import orjson

import bass_rust
import concourse.bass as bass
import concourse.mybir as mybir
import concourse.tile as tile
from concourse.bass_utils import run_bass_kernel_spmd

B, L, N, W = 256, 160, 30, 60
NCORES = 8
BC = B // NCORES          # samples per core
ALPHA = 2e-4
KEXP = 1.0 / (W * W * ALPHA)          # exponent scale in cell units: 1/0.72
SDERF = math.sqrt(KEXP)               # Derivative_Erf input scale
DERF_FIX = math.pi / 4.0              # undo (2/sqrt(pi))^2 from Derivative_Erf
CHUNKS = 40                           # 4 curves x 30 samples per chunk
PTS = 128                             # chunk partition dim: p = 32*lg + n
CW = 60                               # band width (= W)
CCOL = 2 * CHUNKS                     # chunk-coord columns per point row
BAND = CCOL * CW                      # 4800 band elements per partition
RQ = 256.0                            # fixed-point scale for r (1/256 cell)
DEAD_OFF = 64                         # iota offset that kills dead rows

LAST_RESULTS = None  # test harness reads profiling info from here


def _basis_T() -> np.ndarray:
    t = np.arange(N, dtype=np.float32) / np.float32(N)
    t = 2 * t**3 - 3 * t**2 + 2 * t
    t_3_0 = t**3
    t_2_1 = t**2 - t_3_0
    t_1_2 = t_3_0 - 2 * t**2 + t
    t_0_3 = (1 - t) ** 3
    return np.stack([t_3_0, 3 * t_2_1, 3 * t_1_2, t_0_3], axis=1).astype(np.float32)


def _legalize_waits(nc, max_waits: int = 1):
    """Walrus rejects engine instructions carrying more than ~1 sync wait
    ("Too many sync wait commands").  Hoist excess waits onto same-engine
    Drain instructions inserted immediately before the offender."""
    js = orjson.loads(mybir.module_to_json_bytes(nc.m))
    ctr = 0
    for f in js["functions"]:
        for bb in f["blocks"]:
            out = []
            changed = False
            for inst in bb["instructions"]:
                si = inst.get("sync_info")
                waits = si.get("on_wait") if si else None
                if waits and len(waits) > max_waits:
                    keep = waits[:max_waits]
                    for w in waits[max_waits:]:
                        ctr += 1
                        out.append({
                            "debug": inst.get("debug", 0),
                            "engine": inst["engine"],
                            "ins": [], "outs": [],
                            "name": f"waitfix-{ctr}",
                            "opcode": "Drain",
                            "sync_info": {"on_update": [], "on_wait": [w]},
                        })
                    si["on_wait"] = keep
                    changed = True
                out.append(inst)
            if changed:
                bb["instructions"] = out
    if ctr:
        nc.m = bass_rust.module_from_json_bytes(orjson.dumps(js))
    return ctr


def build_program(legalize: bool = True):
    f32 = mybir.dt.float32
    f16 = mybir.dt.float16
    i16 = mybir.dt.int16

    nc = bass.Bass("TRN2", target_bir_lowering=False, debug=False)

    # host-pretransposed input: cols 0:32 hold the (4, 32) stationary basis
    # TscT (r[m] = sum_k TscT[k,m]*ctrl[k] = 60*X), then the control points
    # [k=4, b*cs*g*co] with l = 4*cs+g.
    x_t = nc.dram_tensor("x", [4, 32 + BC * 2 * L], f32, kind="ExternalInput")
    # output in (i, b, j) layout; host transposes back to (b, i, j)
    y_t = nc.dram_tensor("y", [W, BC, W], f32, kind="ExternalOutput")

    # int16 iota seed [128, (i, co)]: val = 256*i, +256*DEAD_OFF on dead rows
    # (n in {30,31} of each 32-strip) so their Gaussian is
    # exp(-1.39*(i+64)^2) = 0 regardless of the y side.  Expanded on-device
    # by a broadcast DVE copy to the interleaved band layout
    # [p, 120*cs + 2*i + co].
    iota_np = np.zeros((PTS, CW, 2), dtype=np.int16)
    iota_np += (RQ * np.arange(CW, dtype=np.float32))[None, :, None].astype(np.int16)
    for lg in range(4):
        iota_np[32 * lg + 30: 32 * lg + 32] += np.int16(int(RQ) * DEAD_OFF)
    iota_np = iota_np.reshape(PTS, 2 * CW)
    iota_d = nc.inline_tensor(iota_np, name="iota16")

    with tile.TileContext(nc) as tc, tc.tile_pool(name="const", bufs=1) as cpool, \
            tc.tile_pool(name="ctrl", bufs=1) as ctrl_pool, \
            tc.tile_pool(name="outp", bufs=1) as out_pool, \
            tc.tile_pool(name="rsb", bufs=6) as rsb_pool, \
            tc.tile_pool(name="band", bufs=3) as band_pool, \
            tc.tile_pool(name="rpsum", bufs=3, space="PSUM") as rps_pool, \
            tc.tile_pool(name="imgpsum", bufs=5, space="PSUM") as img_pool:

        # Prologue: iota seed on the scalar DGE queue; tsc + samples 0-3 of
        # the control points land first on the sync queue so the pipeline
        # starts immediately; the rest trickles in behind.
        iot = cpool.tile([PTS, 2 * CW], i16, tag="iotas")
        nc.scalar.dma_start(iot[:], iota_d.ap())

        # ct lands on only 4 partitions, so per-partition DMA write bandwidth
        # makes one big load finish ~23us in; loading per-4-sample slices in
        # sample order (alternating queues) delivers each r unit's data
        # just in time instead.
        ct = ctrl_pool.tile([4, 32 + BC * 2 * L], f32, tag="ct")
        CT0 = 32 + 4 * 2 * L
        # tsc + sample 0 land first (~0.7us) so the r(0) -> rcopy(0) ->
        # sub(0) chain starts ~1us earlier; samples 1-3 follow on the SAME
        # queue (putting them behind iota on the scalar queue regresses).
        CT00 = 32 + 2 * L
        nc.sync.dma_start(ct[:, :CT00], x_t.ap()[:, :CT00])
        nc.sync.dma_start(ct[:, CT00:CT0], x_t.ap()[:, CT00:CT0])
        for k, c0 in enumerate(range(CT0, 32 + BC * 2 * L, 4 * 2 * L)):
            c1 = c0 + 4 * 2 * L
            eng = nc.scalar if k % 2 == 0 else nc.sync
            eng.dma_start(ct[:, c0:c1], x_t.ap()[:, c0:c1])
        tsc = ct[:, 0:32]
        # view [k, b, cs, g, co]
        ct_view = ct[:, 32:].rearrange("k (b c g co) -> k b c g co",
                                       b=BC, c=CHUNKS, co=2)

        # all 32 output images live here until the per-group DMAs
        out_all = out_pool.tile([W, BC * W], f32, tag="oall")

        GRP = 8
        r_ps_tiles = {}
        r_sb_tiles = {}
        img_tiles = {}

        def emit_r(b0, nb=1):
            """r matmuls for samples [b0, b0+nb) into one PSUM tile
            (fp32, 60*X); one matmul per curve group covers all nb."""
            r_ps = rps_pool.tile([PTS, nb * CCOL], f32, tag="rps")
            for lg in range(4):
                nc.tensor.matmul(
                    r_ps[32 * lg: 32 * lg + 32, :],
                    lhsT=tsc,
                    rhs=ct_view[:, b0: b0 + nb, :, lg: lg + 1, :],
                    start=True,
                    stop=True,
                    tile_position=(0, 32 * lg),
                )
            r_ps_tiles[b0] = (r_ps, nb)

        def emit_rcopy(b0):
            """fixed-point convert r -> int16 (256*r) on DVE."""
            r_ps, nb = r_ps_tiles.pop(b0)
            r_sb = rsb_pool.tile([PTS, nb * CCOL], i16, tag="rsb")
            nc.vector.tensor_scalar(
                r_sb[:], r_ps[:], RQ, None,
                mybir.AluOpType.mult,
            )
            for idx in range(nb):
                r_sb_tiles[b0 + idx] = (r_sb, idx)

        def emit_sub_part(b, dst, doff, c0, nch):
            """band subtract d[p, 120cs+2i+co] = iota16 - 256*r for chunks
            [c0, c0+nch), int16.  All operands 2-byte with step-1 innermost
            (co) -> DVE 2x_1P; iota broadcasts over cs, r over i."""
            r_sb, idx = r_sb_tiles[b]
            base = idx * CCOL
            nc.vector.tensor_tensor(
                dst[:, doff: doff + nch * 2 * CW]
                .rearrange("p (cs i co) -> p cs i co", i=CW, co=2),
                iot[:].rearrange("p (o i co) -> p o i co", o=1, co=2)
                .broadcast_to([PTS, nch, CW, 2]),
                r_sb[:, base + 2 * c0: base + 2 * (c0 + nch)]
                .rearrange("p (cs o co) -> p cs o co", o=1, co=2)
                .broadcast_to([PTS, nch, CW, 2]),
                mybir.AluOpType.subtract,
            )

        def emit_act_part(ddg, off, n):
            """Gaussian on ScalarE, IN PLACE: the f16 output overwrites the
            i16 input via bitcast (the ACT write trails the read by the
            pipeline depth in the same traversal order, so this is race-free
            and halves band SBUF)."""
            src = ddg[:, off: off + n]
            nc.scalar.activation(
                src.bitcast(f16),
                src,
                mybir.ActivationFunctionType.Derivative_Erf,
                bias=0.0, scale=SDERF / RQ,
            )

        def emit_img_part(b, gg, goff, c0, c1):
            """image matmuls for chunks [c0, c1) read from the in-place band
            tile gg (bitcast to f16) at goff (which holds chunks starting at
            c0); PSUM accumulation spans parts."""
            gg_v = gg[:, goff: goff + (c1 - c0) * 2 * CW].bitcast(f16).rearrange(
                "p (cs i co) -> p cs i co", i=CW, co=2)
            if b in img_tiles:
                img = img_tiles[b]
            else:
                img = img_pool.tile([W, W], f32, tag="img")
                img_tiles[b] = img
            for c in range(c0, c1):
                nc.tensor.matmul(
                    img[:],
                    lhsT=gg_v[:, c - c0, :, 0],
                    rhs=gg_v[:, c - c0, :, 1],
                    start=(c == 0),
                    stop=(c == CHUNKS - 1),
                )

        def emit_img(b, gg2, off):
            emit_img_part(b, gg2, off, 0, CHUNKS)

        def emit_min(b, use_scalar=False):
            """min(s*img, 1), then DMA: grouped by 8 in the steady state,
            per-sample in the last group so the tail stays short.  Normally
            on DVE (tensor_scalar mult+min); during the ramp on ScalarE
            (min = 1 - relu(1 - s*img), two activations) because ScalarE is
            gap-waiting there anyway while a DVE min would head-of-line
            block the critical subtracts."""
            img = img_tiles.pop(b)
            if use_scalar:
                tmp = rsb_pool.tile([W, W], f32, tag="mintmp")
                nc.scalar.activation(
                    tmp[:], img[:],
                    mybir.ActivationFunctionType.Relu,
                    bias=1.0, scale=-DERF_FIX,
                )
                nc.scalar.activation(
                    out_all[:, W * b: W * (b + 1)], tmp[:],
                    mybir.ActivationFunctionType.Copy,
                    bias=1.0, scale=-1.0,
                )
            else:
                nc.vector.tensor_scalar(
                    out_all[:, W * b: W * (b + 1)], img[:],
                    DERF_FIX, 1.0,
                    mybir.AluOpType.mult, mybir.AluOpType.min,
                )
            if b >= BC - GRP:
                nc.sync.dma_start(
                    y_t.ap()[:, b: b + 1, :],
                    out_all[:, W * b: W * (b + 1)]
                    .rearrange("i (b j) -> i b j", b=1),
                )
            elif b % GRP == GRP - 1:
                g = b // GRP
                nc.sync.dma_start(
                    y_t.ap()[:, g * GRP: (g + 1) * GRP, :],
                    out_all[:, W * GRP * g: W * GRP * (g + 1)]
                    .rearrange("i (b j) -> i b j", b=GRP),
                )

        # software pipeline over sample groups.  Sample 0 runs as two
        # half-band ACTs so ScalarE starts ~1.3us earlier; sample 31 runs as
        # two half-band ACTs so its first img half overlaps its second ACT;
        # middle samples run in groups of up to 4 per ACT instruction.
        # r matmuls run in units of 4 samples scheduled ~3 groups ahead
        # (singles for 0-3 so the pipeline starts fast); min one group
        # behind.
        HB = BAND // 2
        HC = CHUNKS // 2

        # sample 0 single (fastest possible start), then samples 1-3 as one
        # unit: two DVE rcopies instead of four keeps ~1us of op+drain
        # bubbles out of the critical pipeline-fill window.
        emit_r(0)
        emit_rcopy(0)
        emit_r(1, 3)
        emit_rcopy(1)
        # 4-sample r units, scheduled per iteration index (0 = sample-0
        # block, then one per group): matmuls at first_consumer-4 (so they
        # sit ahead of the blocking img matmuls in the in-order PE queue)
        # and the DVE copies at first_consumer-2 (so they stay out of the
        # DVE's critical pipeline-fill window).
        mm_sched = {0: 4, 2: 8, 4: 12, 5: 16, 6: 20, 7: 24, 9: 28}
        cp_sched = {2: 4, 4: 8, 6: 12, 7: 16, 8: 20, 9: 24, 11: 28}
        it_ctr = {"i": 0}

        def runit_start():
            u = mm_sched.get(it_ctr["i"])
            if u is not None:
                emit_r(u, 4)

        def runit_end():
            u = cp_sched.get(it_ctr["i"])
            if u is not None:
                emit_rcopy(u)
            it_ctr["i"] += 1

        # group-size ramp: singles then pairs, with quads only in the back
        # half where the DVE (2.7us of subtract per sample vs 4.15us of ACT)
        # has built up enough surplus to stay a full quad ahead of ScalarE.
        groups = ([[1], [2], [3]]
                  + [[a, a + 1] for a in range(4, 14, 2)]
                  + [list(range(a, a + 4)) for a in range(14, 26, 4)]
                  + [[26, 27], [28, 29], [BC - 2]])

        def prep_sub(g):
            """allocate a group's band tile and emit its subtracts.  Called
            one iteration AHEAD of the group's ACT, so the ACT's dependency
            is satisfied a full window early and the DVE queue's mins and
            rcopies can never head-of-line block a critical subtract."""
            ddg = band_pool.tile([PTS, len(g) * BAND], i16, tag="dd")
            for idx, b in enumerate(g):
                emit_sub_part(b, ddg, idx * BAND, 0, CHUNKS)
                del r_sb_tiles[b]
            return ddg

        # ---- sample 0: half-band pipeline fill
        runit_start()
        dd0 = band_pool.tile([PTS, BAND], i16, tag="dd")
        emit_sub_part(0, dd0, 0, 0, HC)
        emit_act_part(dd0, 0, HB)
        emit_sub_part(0, dd0, HB, HC, HC)
        del r_sb_tiles[0]
        emit_act_part(dd0, HB, HB)
        emit_img_part(0, dd0, 0, 0, HC)
        emit_img_part(0, dd0, HB, HC, CHUNKS)
        nxt_dd = prep_sub(groups[0])
        runit_end()

        bL = BC - 1
        ddL = None
        pending = [[0]]
        for gi, g in enumerate(groups):
            runit_start()
            ddg = nxt_dd
            emit_act_part(ddg, 0, len(g) * BAND)
            if gi + 1 < len(groups):
                nxt_dd = prep_sub(groups[gi + 1])
            else:
                # sample 31's band, also prepared one window ahead
                ddL = band_pool.tile([PTS, BAND], i16, tag="dd")
                emit_sub_part(bL, ddL, 0, 0, HC)
                emit_sub_part(bL, ddL, HB, HC, HC)
                del r_sb_tiles[bL]
            for idx, b in enumerate(g):
                emit_img(b, ddg, idx * BAND)
            # hold mins two groups back through the ramp (keeps them out of
            # the DVE critical window), one group back in steady state
            pending.append(list(g))
            hold = 2 if it_ctr["i"] <= 4 else 1
            while len(pending) > hold:
                for b in pending.pop(0):
                    emit_min(b)
            runit_end()

        # ---- sample 31: half-band drain (img half overlaps second ACT)
        emit_act_part(ddL, 0, HB)
        emit_img_part(bL, ddL, 0, 0, HC)
        emit_act_part(ddL, HB, HB)
        for pg in pending:
            for b in pg:
                emit_min(b)
        emit_img_part(bL, ddL, HB, HC, CHUNKS)
        emit_min(bL)

    if legalize:
        _legalize_waits(nc)
    return nc


_PROGRAM = None


def kernel(x: np.ndarray, _trace: bool = False) -> np.ndarray:
    global _PROGRAM, LAST_RESULTS
    assert x.shape == (B, L, 8) and x.dtype == np.float32, (x.shape, x.dtype)
    if _PROGRAM is None:
        _PROGRAM = build_program()
    nc = _PROGRAM
    # host-side shard + transpose to k-major: [b, l, (k c)] -> [k, b, cs, g, co],
    # with the (4, 32) basis TscT prepended as the first 32 columns.
    tsc_np = np.zeros((4, 32), dtype=np.float32)
    tsc_np[:, :N] = (W * _basis_T()).T
    xr = np.ascontiguousarray(x).reshape(NCORES, BC, CHUNKS, 4, 4, 2)
    in_maps = []
    for i in range(NCORES):
        xs = xr[i].transpose(3, 0, 1, 2, 4).reshape(4, BC * 2 * L)
        in_maps.append({"x": np.ascontiguousarray(np.concatenate([tsc_np, xs], axis=1))})
    res = run_bass_kernel_spmd(nc, in_maps, list(range(NCORES)), trace=_trace)
    LAST_RESULTS = res
    return np.concatenate(
        [res.results[i]["y"].transpose(1, 0, 2) for i in range(NCORES)], axis=0
    )


# revision 63
# speedup vs baseline: 1.0084x; 1.0084x over previous
"""Bezier-to-image Gaussian splat kernel for Trainium2 (8 NeuronCores).

Reference computation (per sample b of 256):
    T = warped cubic Bernstein basis (30, 4)
    points = einsum('nk,blkc->blnc', T, x.reshape(B,160,4,2))   # (B,160,30,2)
    gx[b,l,i,n] = exp(-(i/60 - X[b,l,n])^2 / 2e-4)
    out[b,i,j]  = min(sum_{l,n} gx[b,l,i,n]*gy[b,l,j,n], 1)     # (B,60,60)

Strategy: pure data parallel, 32 samples per core.  Host pre-transposes
the control points to k-major layout so the input lands in one contiguous
DMA (the old transposing DMA cost ~21k 8-byte descriptors ~ 200us), and
the output DRAM tensor is [W, BC, W] so the store is contiguous too (host
un-permutes).

Per-core pipeline: r = 60*X via tiny fp32 PE matmuls (4-sample units,
scheduled early so the in-order PE queue never blocks them behind image
matmuls); r converts to int16 fixed point (256*r) on DVE; the band
d[p, 120*cs+2*i+co] = 256*(iota - r) is one DVE tensor_tensor subtract
per sample whose operands are all 2-byte with step-1 innermost dims ->
DVE 2x_1P mode (2.6us/sample instead of 5.1); ScalarE (the bottleneck at
1 elem/cycle/lane, ~4.15us/sample) evaluates the Gaussian via
Derivative_Erf(SDERF/256 * d) IN PLACE (f16 over the i16 input via
bitcast - the ACT write trails its read, halving band SBUF) in one
instruction per sample group; the 60x60 image accumulates on PE over 40
interleaved-stride chunk matmuls in one PSUM bank; min(s*img,1) runs on
DVE (tensor_scalar mult+min) to keep ScalarE saturated.

Scheduling: ScalarE group sizes ramp 2x-half, 3x-single, 5x-pair, then
quads (the DVE needs 2.7us of subtract per sample vs ScalarE's 4.15us,
so big ACT groups are only safe once the DVE has built up surplus); each
group's subtracts are emitted one full iteration AHEAD of its ACT so the
ACT dependency is satisfied a window early and the DVE queue's mins and
rcopies can't head-of-line block a critical subtract; the last sample
drains in two half-band ACTs so its first image half overlaps its second
ACT.  Steady state is ~97% ScalarE-occupied; ~154us/kernel = ~13us fill
+ 133us ACT + ~2us gaps + ~6us drain.
"""

import math

import numpy as np
import orjson

import bass_rust
import concourse.bass as bass
import concourse.mybir as mybir
import concourse.tile as tile
from concourse.bass_utils import run_bass_kernel_spmd

B, L, N, W = 256, 160, 30, 60
NCORES = 8
BC = B // NCORES          # samples per core
ALPHA = 2e-4
KEXP = 1.0 / (W * W * ALPHA)          # exponent scale in cell units: 1/0.72
SDERF = math.sqrt(KEXP)               # Derivative_Erf input scale
DERF_FIX = math.pi / 4.0              # undo (2/sqrt(pi))^2 from Derivative_Erf
CHUNKS = 40                           # 4 curves x 30 samples per chunk
PTS = 128                             # chunk partition dim: p = 32*lg + n
CW = 60                               # band width (= W)
CCOL = 2 * CHUNKS                     # chunk-coord columns per point row
BAND = CCOL * CW                      # 4800 band elements per partition
RQ = 256.0                            # fixed-point scale for r (1/256 cell)
DEAD_OFF = 64                         # iota offset that kills dead rows

LAST_RESULTS = None  # test harness reads profiling info from here


def _basis_T() -> np.ndarray:
    t = np.arange(N, dtype=np.float32) / np.float32(N)
    t = 2 * t**3 - 3 * t**2 + 2 * t
    t_3_0 = t**3
    t_2_1 = t**2 - t_3_0
    t_1_2 = t_3_0 - 2 * t**2 + t
    t_0_3 = (1 - t) ** 3
    return np.stack([t_3_0, 3 * t_2_1, 3 * t_1_2, t_0_3], axis=1).astype(np.float32)


def _legalize_waits(nc, max_waits: int = 1):
    """Walrus rejects engine instructions carrying more than ~1 sync wait
    ("Too many sync wait commands").  Hoist excess waits onto same-engine
    Drain instructions inserted immediately before the offender."""
    js = orjson.loads(mybir.module_to_json_bytes(nc.m))
    ctr = 0
    for f in js["functions"]:
        for bb in f["blocks"]:
            out = []
            changed = False
            for inst in bb["instructions"]:
                si = inst.get("sync_info")
                waits = si.get("on_wait") if si else None
                if waits and len(waits) > max_waits:
                    keep = waits[:max_waits]
                    for w in waits[max_waits:]:
                        ctr += 1
                        out.append({
                            "debug": inst.get("debug", 0),
                            "engine": inst["engine"],
                            "ins": [], "outs": [],
                            "name": f"waitfix-{ctr}",
                            "opcode": "Drain",
                            "sync_info": {"on_update": [], "on_wait": [w]},
                        })
                    si["on_wait"] = keep
                    changed = True
                out.append(inst)
            if changed:
                bb["instructions"] = out
    if ctr:
        nc.m = bass_rust.module_from_json_bytes(orjson.dumps(js))
    return ctr


def build_program(legalize: bool = True):
    f32 = mybir.dt.float32
    f16 = mybir.dt.float16
    i16 = mybir.dt.int16

    nc = bass.Bass("TRN2", target_bir_lowering=False, debug=False)

    # host-pretransposed input: cols 0:32 hold the (4, 32) stationary basis
    # TscT (r[m] = sum_k TscT[k,m]*ctrl[k] = 60*X), then the control points
    # [k=4, b*cs*g*co] with l = 4*cs+g.
    x_t = nc.dram_tensor("x", [4, 32 + BC * 2 * L], f32, kind="ExternalInput")
    # output in (i, b, j) layout; host transposes back to (b, i, j)
    y_t = nc.dram_tensor("y", [W, BC, W], f32, kind="ExternalOutput")

    # int16 iota seed [128, (i, co)]: val = 256*i, +256*DEAD_OFF on dead rows
    # (n in {30,31} of each 32-strip) so their Gaussian is
    # exp(-1.39*(i+64)^2) = 0 regardless of the y side.  Expanded on-device
    # by a broadcast DVE copy to the interleaved band layout
    # [p, 120*cs + 2*i + co].
    iota_np = np.zeros((PTS, CW, 2), dtype=np.int16)
    iota_np += (RQ * np.arange(CW, dtype=np.float32))[None, :, None].astype(np.int16)
    for lg in range(4):
        iota_np[32 * lg + 30: 32 * lg + 32] += np.int16(int(RQ) * DEAD_OFF)
    iota_np = iota_np.reshape(PTS, 2 * CW)
    iota_d = nc.inline_tensor(iota_np, name="iota16")

    with tile.TileContext(nc) as tc, tc.tile_pool(name="const", bufs=1) as cpool, \
            tc.tile_pool(name="ctrl", bufs=1) as ctrl_pool, \
            tc.tile_pool(name="outp", bufs=1) as out_pool, \
            tc.tile_pool(name="rsb", bufs=6) as rsb_pool, \
            tc.tile_pool(name="band", bufs=3) as band_pool, \
            tc.tile_pool(name="rpsum", bufs=3, space="PSUM") as rps_pool, \
            tc.tile_pool(name="imgpsum", bufs=5, space="PSUM") as img_pool:

        # Prologue: iota seed on the scalar DGE queue; tsc + samples 0-3 of
        # the control points land first on the sync queue so the pipeline
        # starts immediately; the rest trickles in behind.
        iot = cpool.tile([PTS, 2 * CW], i16, tag="iotas")
        nc.scalar.dma_start(iot[:], iota_d.ap())

        # ct lands on only 4 partitions, so per-partition DMA write bandwidth
        # makes one big load finish ~23us in; loading per-4-sample slices in
        # sample order (alternating queues) delivers each r unit's data
        # just in time instead.
        ct = ctrl_pool.tile([4, 32 + BC * 2 * L], f32, tag="ct")
        CT0 = 32 + 4 * 2 * L
        nc.sync.dma_start(ct[:, :CT0], x_t.ap()[:, :CT0])
        for k, c0 in enumerate(range(CT0, 32 + BC * 2 * L, 4 * 2 * L)):
            c1 = c0 + 4 * 2 * L
            eng = nc.scalar if k % 2 == 0 else nc.sync
            eng.dma_start(ct[:, c0:c1], x_t.ap()[:, c0:c1])
        tsc = ct[:, 0:32]
        # view [k, b, cs, g, co]
        ct_view = ct[:, 32:].rearrange("k (b c g co) -> k b c g co",
                                       b=BC, c=CHUNKS, co=2)

        # all 32 output images live here until the per-group DMAs
        out_all = out_pool.tile([W, BC * W], f32, tag="oall")

        GRP = 8
        r_ps_tiles = {}
        r_sb_tiles = {}
        img_tiles = {}

        def emit_r(b0, nb=1):
            """r matmuls for samples [b0, b0+nb) into one PSUM tile
            (fp32, 60*X); one matmul per curve group covers all nb."""
            r_ps = rps_pool.tile([PTS, nb * CCOL], f32, tag="rps")
            for lg in range(4):
                nc.tensor.matmul(
                    r_ps[32 * lg: 32 * lg + 32, :],
                    lhsT=tsc,
                    rhs=ct_view[:, b0: b0 + nb, :, lg: lg + 1, :],
                    start=True,
                    stop=True,
                    tile_position=(0, 32 * lg),
                )
            r_ps_tiles[b0] = (r_ps, nb)

        def emit_rcopy(b0):
            """fixed-point convert r -> int16 (256*r) on DVE."""
            r_ps, nb = r_ps_tiles.pop(b0)
            r_sb = rsb_pool.tile([PTS, nb * CCOL], i16, tag="rsb")
            nc.vector.tensor_scalar(
                r_sb[:], r_ps[:], RQ, None,
                mybir.AluOpType.mult,
            )
            for idx in range(nb):
                r_sb_tiles[b0 + idx] = (r_sb, idx)

        def emit_sub_part(b, dst, doff, c0, nch):
            """band subtract d[p, 120cs+2i+co] = iota16 - 256*r for chunks
            [c0, c0+nch), int16.  All operands 2-byte with step-1 innermost
            (co) -> DVE 2x_1P; iota broadcasts over cs, r over i."""
            r_sb, idx = r_sb_tiles[b]
            base = idx * CCOL
            nc.vector.tensor_tensor(
                dst[:, doff: doff + nch * 2 * CW]
                .rearrange("p (cs i co) -> p cs i co", i=CW, co=2),
                iot[:].rearrange("p (o i co) -> p o i co", o=1, co=2)
                .broadcast_to([PTS, nch, CW, 2]),
                r_sb[:, base + 2 * c0: base + 2 * (c0 + nch)]
                .rearrange("p (cs o co) -> p cs o co", o=1, co=2)
                .broadcast_to([PTS, nch, CW, 2]),
                mybir.AluOpType.subtract,
            )

        def emit_act_part(ddg, off, n):
            """Gaussian on ScalarE, IN PLACE: the f16 output overwrites the
            i16 input via bitcast (the ACT write trails the read by the
            pipeline depth in the same traversal order, so this is race-free
            and halves band SBUF)."""
            src = ddg[:, off: off + n]
            nc.scalar.activation(
                src.bitcast(f16),
                src,
                mybir.ActivationFunctionType.Derivative_Erf,
                bias=0.0, scale=SDERF / RQ,
            )

        def emit_img_part(b, gg, goff, c0, c1):
            """image matmuls for chunks [c0, c1) read from the in-place band
            tile gg (bitcast to f16) at goff (which holds chunks starting at
            c0); PSUM accumulation spans parts."""
            gg_v = gg[:, goff: goff + (c1 - c0) * 2 * CW].bitcast(f16).rearrange(
                "p (cs i co) -> p cs i co", i=CW, co=2)
            if b in img_tiles:
                img = img_tiles[b]
            else:
                img = img_pool.tile([W, W], f32, tag="img")
                img_tiles[b] = img
            for c in range(c0, c1):
                nc.tensor.matmul(
                    img[:],
                    lhsT=gg_v[:, c - c0, :, 0],
                    rhs=gg_v[:, c - c0, :, 1],
                    start=(c == 0),
                    stop=(c == CHUNKS - 1),
                )

        def emit_img(b, gg2, off):
            emit_img_part(b, gg2, off, 0, CHUNKS)

        def emit_min(b, use_scalar=False):
            """min(s*img, 1), then DMA: grouped by 8 in the steady state,
            per-sample in the last group so the tail stays short.  Normally
            on DVE (tensor_scalar mult+min); during the ramp on ScalarE
            (min = 1 - relu(1 - s*img), two activations) because ScalarE is
            gap-waiting there anyway while a DVE min would head-of-line
            block the critical subtracts."""
            img = img_tiles.pop(b)
            if use_scalar:
                tmp = rsb_pool.tile([W, W], f32, tag="mintmp")
                nc.scalar.activation(
                    tmp[:], img[:],
                    mybir.ActivationFunctionType.Relu,
                    bias=1.0, scale=-DERF_FIX,
                )
                nc.scalar.activation(
                    out_all[:, W * b: W * (b + 1)], tmp[:],
                    mybir.ActivationFunctionType.Copy,
                    bias=1.0, scale=-1.0,
                )
            else:
                nc.vector.tensor_scalar(
                    out_all[:, W * b: W * (b + 1)], img[:],
                    DERF_FIX, 1.0,
                    mybir.AluOpType.mult, mybir.AluOpType.min,
                )
            if b >= BC - GRP:
                nc.sync.dma_start(
                    y_t.ap()[:, b: b + 1, :],
                    out_all[:, W * b: W * (b + 1)]
                    .rearrange("i (b j) -> i b j", b=1),
                )
            elif b % GRP == GRP - 1:
                g = b // GRP
                nc.sync.dma_start(
                    y_t.ap()[:, g * GRP: (g + 1) * GRP, :],
                    out_all[:, W * GRP * g: W * GRP * (g + 1)]
                    .rearrange("i (b j) -> i b j", b=GRP),
                )

        # software pipeline over sample groups.  Sample 0 runs as two
        # half-band ACTs so ScalarE starts ~1.3us earlier; sample 31 runs as
        # two half-band ACTs so its first img half overlaps its second ACT;
        # middle samples run in groups of up to 4 per ACT instruction.
        # r matmuls run in units of 4 samples scheduled ~3 groups ahead
        # (singles for 0-3 so the pipeline starts fast); min one group
        # behind.
        HB = BAND // 2
        HC = CHUNKS // 2

        # sample 0 single (fastest possible start), then samples 1-3 as one
        # unit: two DVE rcopies instead of four keeps ~1us of op+drain
        # bubbles out of the critical pipeline-fill window.
        emit_r(0)
        emit_rcopy(0)
        emit_r(1, 3)
        emit_rcopy(1)
        # 4-sample r units, scheduled per iteration index (0 = sample-0
        # block, then one per group): matmuls at first_consumer-4 (so they
        # sit ahead of the blocking img matmuls in the in-order PE queue)
        # and the DVE copies at first_consumer-2 (so they stay out of the
        # DVE's critical pipeline-fill window).
        mm_sched = {0: 4, 2: 8, 4: 12, 5: 16, 6: 20, 7: 24, 9: 28}
        cp_sched = {2: 4, 4: 8, 6: 12, 7: 16, 8: 20, 9: 24, 11: 28}
        it_ctr = {"i": 0}

        def runit_start():
            u = mm_sched.get(it_ctr["i"])
            if u is not None:
                emit_r(u, 4)

        def runit_end():
            u = cp_sched.get(it_ctr["i"])
            if u is not None:
                emit_rcopy(u)
            it_ctr["i"] += 1

        # group-size ramp: singles then pairs, with quads only in the back
        # half where the DVE (2.7us of subtract per sample vs 4.15us of ACT)
        # has built up enough surplus to stay a full quad ahead of ScalarE.
        groups = ([[1], [2], [3]]
                  + [[a, a + 1] for a in range(4, 14, 2)]
                  + [list(range(a, a + 4)) for a in range(14, 26, 4)]
                  + [[26, 27], [28, 29], [BC - 2]])

        def prep_sub(g):
            """allocate a group's band tile and emit its subtracts.  Called
            one iteration AHEAD of the group's ACT, so the ACT's dependency
            is satisfied a full window early and the DVE queue's mins and
            rcopies can never head-of-line block a critical subtract."""
            ddg = band_pool.tile([PTS, len(g) * BAND], i16, tag="dd")
            for idx, b in enumerate(g):
                emit_sub_part(b, ddg, idx * BAND, 0, CHUNKS)
                del r_sb_tiles[b]
            return ddg

        # ---- sample 0: half-band pipeline fill
        runit_start()
        dd0 = band_pool.tile([PTS, BAND], i16, tag="dd")
        emit_sub_part(0, dd0, 0, 0, HC)
        emit_act_part(dd0, 0, HB)
        emit_sub_part(0, dd0, HB, HC, HC)
        del r_sb_tiles[0]
        emit_act_part(dd0, HB, HB)
        emit_img_part(0, dd0, 0, 0, HC)
        emit_img_part(0, dd0, HB, HC, CHUNKS)
        nxt_dd = prep_sub(groups[0])
        runit_end()

        bL = BC - 1
        ddL = None
        pending = [[0]]
        for gi, g in enumerate(groups):
            runit_start()
            ddg = nxt_dd
            emit_act_part(ddg, 0, len(g) * BAND)
            if gi + 1 < len(groups):
                nxt_dd = prep_sub(groups[gi + 1])
            else:
                # sample 31's band, also prepared one window ahead
                ddL = band_pool.tile([PTS, BAND], i16, tag="dd")
                emit_sub_part(bL, ddL, 0, 0, HC)
                emit_sub_part(bL, ddL, HB, HC, HC)
                del r_sb_tiles[bL]
            for idx, b in enumerate(g):
                emit_img(b, ddg, idx * BAND)
            # hold mins two groups back through the ramp (keeps them out of
            # the DVE critical window), one group back in steady state
            pending.append(list(g))
            hold = 2 if it_ctr["i"] <= 4 else 1
            while len(pending) > hold:
                for b in pending.pop(0):
                    emit_min(b)
            runit_end()

        # ---- sample 31: half-band drain (img half overlaps second ACT)
        emit_act_part(ddL, 0, HB)
        emit_img_part(bL, ddL, 0, 0, HC)
        emit_act_part(ddL, HB, HB)
        for pg in pending:
            for b in pg:
                emit_min(b)
        emit_img_part(bL, ddL, HB, HC, CHUNKS)
        emit_min(bL)

    if legalize:
        _legalize_waits(nc)
    return nc


_PROGRAM = None


def kernel(x: np.ndarray, _trace: bool = False) -> np.ndarray:
    global _PROGRAM, LAST_RESULTS
    assert x.shape == (B, L, 8) and x.dtype == np.float32, (x.shape, x.dtype)
    if _PROGRAM is None:
        _PROGRAM = build_program()
    nc = _PROGRAM
    # host-side shard + transpose to k-major: [b, l, (k c)] -> [k, b, cs, g, co],
    # with the (4, 32) basis TscT prepended as the first 32 columns.
    tsc_np = np.zeros((4, 32), dtype=np.float32)
    tsc_np[:, :N] = (W * _basis_T()).T
    xr = np.ascontiguousarray(x).reshape(NCORES, BC, CHUNKS, 4, 4, 2)
    in_maps = []
    for i in range(NCORES):
        xs = xr[i].transpose(3, 0, 1, 2, 4).reshape(4, BC * 2 * L)
        in_maps.append({"x": np.ascontiguousarray(np.concatenate([tsc_np, xs], axis=1))})
    res = run_bass_kernel_spmd(nc, in_maps, list(range(NCORES)), trace=_trace)
    LAST_RESULTS = res
    return np.concatenate(
        [res.results[i]["y"].transpose(1, 0, 2) for i in range(NCORES)], axis=0
    )
